# revision 14
# baseline (speedup 1.0000x reference)
"""DecoderRNN (show-attend-tell) Trainium2 kernel.

Strategy: pure data-parallel over batch — each of the 8 NeuronCores runs
8 of the 64 captions end-to-end with all weights replicated, so there is
no cross-core traffic at all. Within a core:

  - att1 = feats @ We.T is precomputed once (bf16 matmuls, fp32 psum)
  - the 20 decode steps run the attention + LSTM recurrence
  - the vocab projection (fc) over all 20 steps is deferred to one
    dense matmul block at the end (it does not feed the recurrence)

The batch sort/unsort in the reference is a mathematical no-op (every
op is batch-elementwise), so caption_lengths is unused. The embedding
gather (64x20 rows) and all weight-layout transposes / bf16 casts are
host-side input prep; all FLOPs run on device.
"""

import numpy as np
import ml_dtypes

import concourse.bass as bass
import concourse.mybir as mybir
import concourse.tile as tile
from concourse.bass_utils import run_bass_kernel_spmd
from concourse.masks import make_identity

BF16 = ml_dtypes.bfloat16

B, T, P, ENC, E, H, A, V = 64, 20, 196, 2048, 512, 512, 512, 10000
NCORES = 8
BL = B // NCORES            # local batch per core = 8
BP = BL * P                 # 1568
VT = 79                     # ceil(V/128) vocab tiles
VPAD = VT * 128             # 10112
BT = BL * T                 # 160  (bt = t*BL + b)

F32 = mybir.dt.float32
BF = mybir.dt.bfloat16
AF = mybir.ActivationFunctionType
ALU = mybir.AluOpType
AX = mybir.AxisListType

_CACHE = {}


def _legalize_multiwaits(nc):
    """This walrus build encodes one semaphore wait per instruction;
    Tile attaches whole residual wait-sets to single instructions.
    Split extra waits onto same-engine no-ops placed just before."""
    f = nc.m.functions[0]
    for b in f.blocks:
        new_list = []
        for inst in b.instructions:
            si = inst.sync_info
            if si is not None and si.on_wait and len(si.on_wait) > 1:
                waits = list(si.on_wait)
                for j, w in enumerate(waits[:-1]):
                    nop = mybir.InstNoOp(name=f"{inst.name}-hw{j}", ins=[], outs=[])
                    nop.engine = inst.engine
                    nop.sync_info = mybir.SyncInfo(on_wait=[w], on_update=[])
                    new_list.append(nop)
                inst.sync_info = mybir.SyncInfo(
                    on_wait=[waits[-1]], on_update=list(si.on_update or [])
                )
            if si is not None and si.on_update and len(si.on_update) > 1:
                raise RuntimeError(f"multi-update on {inst.name}")
            new_list.append(inst)
        b.instructions = new_list


def _build_nc():
    nc = bass.Bass()

    # ---- DRAM parameters (per-core inputs; weights replicated) ----
    d_featsT = nc.declare_dram_parameter("featsT", [128, 16, BP], BF, isOutput=False)
    d_fpad = nc.declare_dram_parameter("feats_pad", [128, 16, ENC], BF, isOutput=False)
    d_embT = nc.declare_dram_parameter("embT", [128, T, 4, BL], BF, isOutput=False)
    d_WeT = nc.declare_dram_parameter("WeT", [128, 16, A], BF, isOutput=False)
    d_WdT = nc.declare_dram_parameter("WdT", [128, 4, A], BF, isOutput=False)
    d_WihT = nc.declare_dram_parameter("WihT", [128, 20, 2048], BF, isOutput=False)
    d_WhhT = nc.declare_dram_parameter("WhhT", [128, 4, 2048], BF, isOutput=False)
    d_fcWT = nc.declare_dram_parameter("fcWT", [128, 4, VPAD], BF, isOutput=False)
    d_wf = nc.declare_dram_parameter("wf_col", [128, 4], BF, isOutput=False)
    d_be = nc.declare_dram_parameter("be_col", [128, 4], F32, isOutput=False)
    d_bd = nc.declare_dram_parameter("bd_col", [128, 4], F32, isOutput=False)
    d_gb = nc.declare_dram_parameter("gbias", [128, 16], F32, isOutput=False)
    d_fcb = nc.declare_dram_parameter("fcb_col", [128, VT], F32, isOutput=False)
    d_logits = nc.declare_dram_parameter("logits_sh", [VT, 128, BT], BF, isOutput=True)
    d_alphas = nc.declare_dram_parameter("alphas_sh", [BL, T, P], F32, isOutput=True)

    with tile.TileContext(nc) as tc:
        with (
            tc.tile_pool(name="singles", bufs=1) as sg,
            tc.tile_pool(name="work", bufs=2) as wp,
            tc.tile_pool(name="work1", bufs=1) as wp1,
            tc.tile_pool(name="psum", bufs=2, space="PSUM") as pp,
        ):
            # ---- persistent SBUF state ----
            ident_f = sg.tile([128, 128], F32, tag="identf")
            make_identity(nc, ident_f)
            ident_b = sg.tile([128, 128], BF, tag="identb")
            make_identity(nc, ident_b)

            wf_sb = sg.tile([128, 4], BF, tag="wf")
            nc.sync.dma_start(out=wf_sb, in_=d_wf[:])
            be_col = sg.tile([128, 4], F32, tag="be")
            nc.sync.dma_start(out=be_col, in_=d_be[:])
            bd_col = sg.tile([128, 4], F32, tag="bd")
            nc.sync.dma_start(out=bd_col, in_=d_bd[:])
            gbias = sg.tile([128, 16], F32, tag="gb")
            nc.sync.dma_start(out=gbias, in_=d_gb[:])
            fcb_col = sg.tile([128, VT], F32, tag="fcb")
            nc.sync.dma_start(out=fcb_col, in_=d_fcb[:])
            WdT = sg.tile([128, 4, A], BF, tag="WdT")
            nc.sync.dma_start(out=WdT, in_=d_WdT[:])
            WhhT = sg.tile([128, 4, 2048], BF, tag="WhhT")
            nc.sync.dma_start(out=WhhT, in_=d_WhhT[:])
            embT = sg.tile([128, T, 4, BL], BF, tag="embT")
            nc.sync.dma_start(out=embT, in_=d_embT[:])

            att1 = sg.tile([128, 4, BP], BF, tag="att1")      # [a_lane, a_tile, bp]
            alpha_bd = sg.tile([128, 16, BL], BF, tag="abd")  # block-diag alpha
            nc.vector.memset(alpha_bd, 0.0)
            hT_bf = sg.tile([128, 4, BL], BF, tag="hT")       # [h_lane, h_tile, b]
            nc.vector.memset(hT_bf, 0.0)
            c_sb = sg.tile([128, 4, BL], F32, tag="c")
            nc.vector.memset(c_sb, 0.0)
            h_hist = sg.tile([128, 4, T, BL], BF, tag="hhist")

            # ---- phase 1: att1 = feats @ We.T + be  (att1_T layout) ----
            with tc.tile_pool(name="p_att1", bufs=1) as pa:
                featsT = pa.tile([128, 16, BP], BF, tag="featsT")
                nc.sync.dma_start(out=featsT, in_=d_featsT[:])
                WeT = pa.tile([128, 16, A], BF, tag="WeT")
                nc.sync.dma_start(out=WeT, in_=d_WeT[:])
                CH = 392  # bp chunk (psum: 392 fp32 = 1568B <= 2KB bank)
                for mt in range(4):
                    for ci in range(BP // CH):
                        ps = pp.tile([128, CH], F32, tag="work")
                        for kt in range(16):
                            nc.tensor.matmul(
                                ps,
                                lhsT=WeT[:, kt, mt * 128:(mt + 1) * 128],
                                rhs=featsT[:, kt, ci * CH:(ci + 1) * CH],
                                start=(kt == 0),
                                stop=(kt == 15),
                            )
                        nc.vector.tensor_scalar(
                            out=att1[:, mt, ci * CH:(ci + 1) * CH],
                            in0=ps,
                            scalar1=be_col[:, mt:mt + 1],
                            scalar2=None,
                            op0=ALU.add,
                        )

            # ---- phase 2: the 20 decode steps ----
            pl_cm = tc.tile_pool(name="p_loop", bufs=1)
            pl = pl_cm.__enter__()
            fpad = pl.tile([128, 16, ENC], BF, tag="fpad")
            nc.sync.dma_start(out=fpad, in_=d_fpad[:])
            WihT = pl.tile([128, 20, 2048], BF, tag="WihT")
            nc.sync.dma_start(out=WihT, in_=d_WihT[:])

            att1_v = att1.rearrange("l m (b p) -> l m b p", b=BL)

            for t in range(T):
                # (a) att2 = Wd @ h (+bd)   [a_lane, a_tile, b]
                ps_a2 = pp.tile([128, 4, BL], F32, tag="att2")
                for mt in range(4):
                    for kt in range(4):
                        nc.tensor.matmul(
                            ps_a2[:, mt, :],
                            lhsT=WdT[:, kt, mt * 128:(mt + 1) * 128],
                            rhs=hT_bf[:, kt, :],
                            start=(kt == 0),
                            stop=(kt == 3),
                        )
                att2 = wp.tile([128, 4, BL], F32, tag="att2s")
                for mt in range(4):
                    nc.vector.tensor_scalar(
                        out=att2[:, mt, :], in0=ps_a2[:, mt, :],
                        scalar1=bd_col[:, mt:mt + 1], scalar2=None, op0=ALU.add,
                    )

                # (b) r = relu(att1 + att2)  — fused add+max on DVE, 4x mode
                r_sb = wp1.tile([128, 4, BL, P], BF, tag="r")
                for mt in range(4):
                    for b in range(BL):
                        nc.vector.tensor_scalar(
                            out=r_sb[:, mt, b, :],
                            in0=att1_v[:, mt, b, :],
                            scalar1=att2[:, mt, b:b + 1],
                            scalar2=0.0,
                            op0=ALU.add,
                            op1=ALU.max,
                        )

                # (c) e[b,p] = wf . r[:, b, p] — computed transposed
                # (r chunks stationary, wf moving), then PE-transposed back,
                # because compute engines cannot address partition base b.
                ps_eT = pp.tile([128, 2, BL], F32, tag="work")
                for b in range(BL):
                    for half, (o0, w) in enumerate(((0, 128), (128, 68))):
                        for ta in range(4):
                            nc.tensor.matmul(
                                ps_eT[0:w, half, b:b + 1],
                                lhsT=r_sb[:, ta, b, o0:o0 + w],
                                rhs=wf_sb[:, ta:ta + 1],
                                start=(ta == 0),
                                stop=(ta == 3),
                            )
                eT_sb = wp.tile([128, 2, BL], F32, tag="eT")
                nc.vector.tensor_copy(eT_sb, ps_eT)
                ps_e2 = pp.tile([BL, P], F32, tag="work")
                nc.tensor.transpose(
                    ps_e2[:, 0:128], eT_sb[:, 0, :], ident_f[:, :]
                )
                nc.tensor.transpose(
                    ps_e2[:, 128:196], eT_sb[0:68, 1, :], ident_f[0:68, 0:68]
                )
                e_sb = wp.tile([BL, P], F32, tag="e")
                nc.vector.tensor_copy(e_sb, ps_e2)

                # (d) softmax over p (bf shift is softmax-invariant)
                mx = wp.tile([BL, 1], F32, tag="mx")
                nc.vector.tensor_reduce(
                    out=mx, in_=e_sb, axis=AX.X, op=ALU.max
                )
                nmx = wp.tile([BL, 1], F32, tag="nmx")
                nc.vector.tensor_scalar_mul(nmx, mx, -1.0)
                ex = wp.tile([BL, P], F32, tag="ex")
                sm = wp.tile([BL, 1], F32, tag="sm")
                nc.scalar.activation(
                    out=ex, in_=e_sb, func=AF.Exp, bias=nmx, scale=1.0,
                    accum_out=sm,
                )
                rs = wp.tile([BL, 1], F32, tag="rs")
                nc.vector.reciprocal(rs, sm)
                alpha = wp.tile([BL, P], F32, tag="alpha")
                nc.vector.tensor_scalar(
                    out=alpha, in0=ex, scalar1=rs, scalar2=None, op0=ALU.mult
                )
                nc.sync.dma_start(out=d_alphas[:, t, :], in_=alpha)

                # (e) transpose alpha into the block-diagonal (padded p-major)
                ps_aT = pp.tile([128, 2, BL], F32, tag="work")
                nc.tensor.transpose(
                    ps_aT[:, 0, :], alpha[:, 0:128], ident_f[0:BL, 0:BL]
                )
                nc.tensor.transpose(
                    ps_aT[0:68, 1, :], alpha[:, 128:196], ident_f[0:BL, 0:BL]
                )
                # scatter columns into diag strips: abd[:, 2b+half, b] <- ps_aT[:, half, b]
                abd_view = bass.AP(
                    tensor=alpha_bd.tensor,
                    offset=alpha_bd.offset,
                    ap=[alpha_bd.ap[0], [17, BL], [8, 2]],
                )
                src_view = bass.AP(
                    tensor=ps_aT.tensor,
                    offset=ps_aT.offset,
                    ap=[ps_aT.ap[0], [1, BL], [8, 2]],
                )
                nc.vector.tensor_copy(abd_view, src_view)

                # (f) ctx[b, :] = sum_p alpha[b,p] feats[b,p,:]
                ctx_bf = wp1.tile([BL, 4, 512], BF, tag="ctx")
                for ci in range(4):
                    ps_c = pp.tile([BL, 512], F32, tag="work")
                    for kt in range(16):
                        nc.tensor.matmul(
                            ps_c,
                            lhsT=alpha_bd[:, kt, :],
                            rhs=fpad[:, kt, ci * 512:(ci + 1) * 512],
                            start=(kt == 0),
                            stop=(kt == 15),
                        )
                    nc.vector.tensor_copy(ctx_bf[:, ci, :], ps_c)

                # (g) transpose ctx -> xT  [x_lane, x_tile, b]
                ps_xT = pp.tile([128, 16, BL], BF, tag="xT")
                for i in range(16):
                    nc.tensor.transpose(
                        ps_xT[:, i, :],
                        ctx_bf[:, i // 4, (i % 4) * 128:(i % 4 + 1) * 128],
                        ident_b[0:BL, 0:BL],
                    )
                xT_bf = wp.tile([128, 16, BL], BF, tag="xTs")
                nc.vector.tensor_copy(xT_bf, ps_xT)

                # (h) gates = W_ih @ [ctx; emb] + W_hh @ h   [g_lane, g_tile, b]
                ps_g = pp.tile([128, 16, BL], F32, tag="g")
                for mt in range(16):
                    for kt in range(20):
                        rhs = xT_bf[:, kt, :] if kt < 16 else embT[:, t, kt - 16, :]
                        nc.tensor.matmul(
                            ps_g[:, mt, :],
                            lhsT=WihT[:, kt, mt * 128:(mt + 1) * 128],
                            rhs=rhs,
                            start=(kt == 0),
                            stop=False,
                        )
                    for kt in range(4):
                        nc.tensor.matmul(
                            ps_g[:, mt, :],
                            lhsT=WhhT[:, kt, mt * 128:(mt + 1) * 128],
                            rhs=hT_bf[:, kt, :],
                            start=False,
                            stop=(kt == 3),
                        )

                # (i) gate nonlinearities via tanh (sigmoid(z)=0.5*tanh(z/2)+0.5)
                th = wp.tile([128, 16, BL], F32, tag="th")
                for mt in range(16):
                    nc.scalar.activation(
                        out=th[:, mt, :], in_=ps_g[:, mt, :], func=AF.Tanh,
                        bias=gbias[:, mt:mt + 1],
                        scale=(1.0 if (8 <= mt < 12) else 0.5),
                    )

                # (j) LSTM pointwise (fp32)
                sigf = wp.tile([128, 4, BL], F32, tag="sigf")
                nc.vector.tensor_scalar(
                    out=sigf, in0=th[:, 4:8, :], scalar1=0.5, scalar2=0.5,
                    op0=ALU.mult, op1=ALU.add,
                )
                nc.vector.tensor_tensor(
                    out=c_sb, in0=sigf, in1=c_sb, op=ALU.mult
                )
                sigi = wp.tile([128, 4, BL], F32, tag="sigi")
                nc.vector.tensor_scalar(
                    out=sigi, in0=th[:, 0:4, :], scalar1=0.5, scalar2=0.5,
                    op0=ALU.mult, op1=ALU.add,
                )
                term = wp.tile([128, 4, BL], F32, tag="term")
                nc.vector.tensor_tensor(
                    out=term, in0=sigi, in1=th[:, 8:12, :], op=ALU.mult
                )
                nc.vector.tensor_tensor(out=c_sb, in0=c_sb, in1=term, op=ALU.add)
                tanc = wp.tile([128, 4, BL], F32, tag="tanc")
                nc.scalar.activation(out=tanc, in_=c_sb, func=AF.Tanh)
                sigo = wp.tile([128, 4, BL], F32, tag="sigo")
                nc.vector.tensor_scalar(
                    out=sigo, in0=th[:, 12:16, :], scalar1=0.5, scalar2=0.5,
                    op0=ALU.mult, op1=ALU.add,
                )
                hf = wp.tile([128, 4, BL], F32, tag="hf")
                nc.vector.tensor_tensor(out=hf, in0=sigo, in1=tanc, op=ALU.mult)
                nc.vector.tensor_copy(hT_bf, hf)
                nc.vector.tensor_copy(h_hist[:, :, t, :], hT_bf)

            pl_cm.__exit__(None, None, None)

            # ---- phase 3: fc over all (t, b) ----
            with tc.tile_pool(name="p_fc", bufs=1) as pf:
                fcWT = pf.tile([128, 4, VPAD], BF, tag="fcWT")
                nc.sync.dma_start(out=fcWT, in_=d_fcWT[:])
                hh = h_hist.rearrange("l k t b -> l k (t b)")
                for vt in range(VT):
                    ps_f = pp.tile([128, BT], F32, tag="work")
                    for kt in range(4):
                        nc.tensor.matmul(
                            ps_f,
                            lhsT=fcWT[:, kt, vt * 128:(vt + 1) * 128],
                            rhs=hh[:, kt, :],
                            start=(kt == 0),
                            stop=(kt == 3),
                        )
                    fo = wp.tile([128, BT], BF, tag="fo")
                    nc.vector.tensor_scalar(
                        out=fo, in0=ps_f, scalar1=fcb_col[:, vt:vt + 1],
                        scalar2=None, op0=ALU.add,
                    )
                    nc.sync.dma_start(out=d_logits[vt], in_=fo)

    _legalize_multiwaits(nc)
    return nc


def _prep_inputs(inputs):
    """Host-side: shard batch, transpose/cast weights into device layouts."""
    f32 = np.float32
    feats = np.asarray(inputs["features"], f32)
    caps = np.asarray(inputs["captions"])
    embW = np.asarray(inputs["embed_W"], f32)
    We = np.asarray(inputs["We"], f32)
    be = np.asarray(inputs["be"], f32)
    Wd = np.asarray(inputs["Wd"], f32)
    bd = np.asarray(inputs["bd"], f32)
    wf = np.asarray(inputs["wf"], f32)
    W_ih = np.asarray(inputs["W_ih"], f32)
    W_hh = np.asarray(inputs["W_hh"], f32)
    b_ih = np.asarray(inputs["b_ih"], f32)
    b_hh = np.asarray(inputs["b_hh"], f32)
    fc_W = np.asarray(inputs["fc_W"], f32)
    fc_b = np.asarray(inputs["fc_b"], f32)

    def kT(M, kt):  # [Ktot, N] -> [128, kt, N]
        Ktot, N = M.shape
        return np.ascontiguousarray(
            M.reshape(kt, 128, N).transpose(1, 0, 2)
        )

    WeT = kT(We.T, 16).astype(BF16)                       # [128,16,512]
    WdT = kT(Wd.T, 4).astype(BF16)                        # [128,4,512]
    WihT = kT(W_ih.T, 20).astype(BF16)                    # [128,20,2048]
    WhhT = kT(W_hh.T, 4).astype(BF16)                     # [128,4,2048]
    fcP = np.zeros((H, VPAD), f32)
    fcP[:, :V] = fc_W.T
    fcWT = kT(fcP, 4).astype(BF16)                        # [128,4,10112]
    wf_col = np.ascontiguousarray(wf[0].reshape(4, 128).T).astype(BF16)
    be_col = np.ascontiguousarray(be.reshape(4, 128).T)
    bd_col = np.ascontiguousarray(bd.reshape(4, 128).T)
    gb = b_ih + b_hh
    gbias = np.ascontiguousarray(gb.reshape(16, 128).T).copy()
    for mt in range(16):
        if not (8 <= mt < 12):      # i, f, o gates use tanh(z/2 + b/2)
            gbias[:, mt] *= 0.5
    fcbP = np.zeros(VPAD, f32)
    fcbP[:V] = fc_b
    fcb_col = np.ascontiguousarray(fcbP.reshape(VT, 128).T)

    emb = embW[caps]                                      # [B,T,E] host gather

    in_maps = []
    for ci in range(NCORES):
        s = slice(ci * BL, (ci + 1) * BL)
        fs = feats[s]                                     # [8,196,2048]
        fm = fs.reshape(BP, ENC)
        featsT = np.ascontiguousarray(
            fm.T.reshape(16, 128, BP).transpose(1, 0, 2)
        ).astype(BF16)
        fpad = np.zeros((128, 16, ENC), BF16)
        for b in range(BL):
            fpad[:, 2 * b, :] = fs[b, 0:128, :]
            fpad[0:68, 2 * b + 1, :] = fs[b, 128:196, :]
        em = emb[s]                                       # [8,20,512]
        embT = np.ascontiguousarray(
            em.transpose(2, 1, 0).reshape(4, 128, T, BL).transpose(1, 2, 0, 3)
        ).astype(BF16)
        in_maps.append(dict(
            featsT=featsT, feats_pad=fpad, embT=embT,
            WeT=WeT, WdT=WdT, WihT=WihT, WhhT=WhhT, fcWT=fcWT,
            wf_col=wf_col, be_col=be_col, bd_col=bd_col,
            gbias=gbias, fcb_col=fcb_col,
        ))
    return in_maps


def kernel(**inputs):
    if "nc" not in _CACHE:
        _CACHE["nc"] = _build_nc()
    nc = _CACHE["nc"]
    in_maps = _prep_inputs(inputs)
    res = run_bass_kernel_spmd(nc, in_maps, core_ids=list(range(NCORES)))
    _CACHE["last_result"] = res

    logits = np.empty((B, T, V), np.float32)
    alphas = np.empty((B, T, P), np.float32)
    for ci in range(NCORES):
        lo = np.asarray(res.results[ci]["logits_sh"]).astype(np.float32)
        lo = lo.reshape(VPAD, T, BL)[:V]                  # [V, t, b]
        logits[ci * BL:(ci + 1) * BL] = lo.transpose(2, 1, 0)
        alphas[ci * BL:(ci + 1) * BL] = np.asarray(res.results[ci]["alphas_sh"])
    return logits, alphas
